# revision 63
# baseline (speedup 1.0000x reference)
"""Trainium2 Bass kernel for a 16-head MHA layer (batch 4, seq 2048, embed 1024).

Sharding: 8 cores; core c handles batch c//2 and query-token half c%2.
Each core receives its batch's x rotated so that its 1024 query tokens sit in
rows 0:1024 (softmax/attention is permutation-invariant over key order, so the
rotation changes nothing mathematically). K/V are computed over the full
sequence on-core, so no collectives are needed. Weights are replicated.

Host-side prep (layout/dtype only): x is passed transposed twice - bf16
panels (xTb[p, ee, t] = x[t, 128*ee+p]) for the V projection, and an
fp8 e-dim-folded copy (xT8[p, a, h, t] = x[t, 256a+128h+p]) for the Q/K
projections; W_qk is fp8 pre-scaled by 256 in the matching fold; W_v and
W_o are bf16 panels. All fp8 is e4m3.

Compute structure per core:
 - Q/K projections as fp8 DoubleRow matmuls (contraction 256/step); the
   1/256 weight prescale is undone in the fused stage op (PSUM -> fp8
   stage with scale+bias on DVE). V projection stays bf16: V-side noise
   passes straight into the output, while Q/K noise only perturbs
   softmax weights, so fp8 is affordable only for Q/K.
 - Scores run as fp8 DoubleRow matmuls over the head dim (d=64 folded as
   32 partitions x 2), K stationary, 512 q per step.
 - exp() is split across two engines: most tiles on the scalar engine,
   a subset on DVE via an integer Schraudolph trick: n = round(a*s + b)
   written as int16 and bit-reinterpreted as bf16 approximates exp(s/8)
   to ~1.8% rms. Softmax normalizes away shared scale, so only relative
   weight noise matters.
 - AV is bf16; P^T is the stationary operand and V streams 64+1 columns
   (the ones-column produces the softmax denominator in the same
   matmul). AV for key-tile kt is emitted one slot late (after the
   scores of kt+1) so its wait on exp(kt) never head-of-line blocks the
   next scores in the in-order PE queue.
 - Per (head, q-tile): reciprocal of the denominator + fused scale on DVE,
   then a DMA-transpose flips the pair back to [d, q] for the out-proj.
 - V-bias folds through softmax (sum(attn)=1) into the out-proj bias.
"""

import sys

for _p in ("/opt/trn_rl_repo",):
    if _p not in sys.path:
        sys.path.insert(0, _p)

import numpy as np

import concourse.bass as bass  # noqa: E402
import concourse.mybir as mybir  # noqa: E402
import concourse.tile as tile  # noqa: E402
from concourse import bacc  # noqa: E402

SEQ = 2048
E = 1024
H = 16
D = 64
NQ = 1024  # query tokens per core
N_CORES = 8

TT = SEQ // 128  # 16 token tiles
KT = SEQ // 128  # 16 key tiles
QT = NQ // 128   # 8 query tiles

F32 = mybir.dt.float32
BF16 = mybir.dt.bfloat16
F8 = mybir.dt.float8e4
I16 = mybir.dt.int16
AF = mybir.ActivationFunctionType
DR = mybir.MatmulPerfMode.DoubleRow
ALU = mybir.AluOpType

WSCALE = 256.0               # W_qk prescale (keeps fp8 weights out of denormals)
S16_MUL = 23.083120654223414  # 128 * log2(e) * 0.125
S16_ADD = 16248.66            # 128*(127 - E[log2(1+t)-t]): mean-unbiased
DVE_KT = (5, 7, 9, 11, 13)    # key tiles whose exp runs on DVE (rest: scalar)


def build_program():
    nc = bacc.Bacc(trn_type="TRN2", target_bir_lowering=False, debug=False)

    xTb = nc.dram_tensor("xTb", [128, 8, SEQ], BF16, kind="ExternalInput").ap()
    w8 = nc.dram_tensor("w8", [128, 4, 2, 2 * E], F8, kind="ExternalInput").ap()
    wvbf = nc.dram_tensor("wvbf", [128, 8, E], BF16, kind="ExternalInput").ap()
    wobf = nc.dram_tensor("wobf", [128, 8, E], BF16, kind="ExternalInput").ap()
    bqkv = nc.dram_tensor("bqkv", [3 * E], F32, kind="ExternalInput").ap()
    bo = nc.dram_tensor("bo", [E], F32, kind="ExternalInput").ap()
    out = nc.dram_tensor("out", [NQ, E], F32, kind="ExternalOutput").ap()

    with tile.TileContext(nc) as tc:
        _body(nc, tc, xTb, w8, wvbf, wobf, bqkv, bo, out)

    nc.compile()
    return nc


def _body(nc, tc, xTbd, w8d, wvbfd, wobfd, bqkv, bo, out):
    from contextlib import ExitStack

    es = ExitStack()
    with es:
        pc = es.enter_context(tc.tile_pool(name="const", bufs=1))
        pbig = es.enter_context(tc.tile_pool(name="big", bufs=1))
        pprt = es.enter_context(tc.tile_pool(name="part", bufs=1))
        pkstg = es.enter_context(tc.tile_pool(name="kstg", bufs=1))
        pqstg = es.enter_context(tc.tile_pool(name="qstg", bufs=1))
        ppt = es.enter_context(tc.tile_pool(name="pt", bufs=4))
        pasb = es.enter_context(tc.tile_pool(name="asb", bufs=10))
        prec = es.enter_context(tc.tile_pool(name="rec", bufs=4))
        pps = es.enter_context(tc.tile_pool(name="ps_s", bufs=2, space="PSUM"))
        ppj = es.enter_context(tc.tile_pool(name="ps_pj", bufs=2, space="PSUM"))
        pav = es.enter_context(tc.tile_pool(name="ps_av", bufs=2, space="PSUM"))

        # xT8/w8 die after the last Q/K chain (window 10); wobf reuses
        # their space (the "late" pool opens after pearly closes).
        pearly_es = ExitStack()
        pearly = pearly_es.enter_context(tc.tile_pool(name="early", bufs=1))

        # --- persistent SBUF tensors -----------------------------------
        # xT8[p, ee, t] = fp8(xTb[p, ee, t]): the e-dim fold for DoubleRow is
        # (p, ee parity), so the fp8 copy shares xTb's layout exactly.
        xT8 = pearly.tile([128, 8, SEQ], F8, tag="xT8")
        w8 = pearly.tile([128, 4, 2, 2 * E], F8, tag="w8")
        xTb = pbig.tile([128, 8, SEQ], BF16, tag="xTb")
        wvbf = pbig.tile([128, 8, E], BF16, tag="wvbf")
        wobf_box = {}
        # fp8 DoubleRow K/Q: tile g holds heads 4g..4g+3 at partition blocks
        # 32a; free dims = [d-half, token]
        K8 = [pbig.tile([128, 2, SEQ], F8, tag=f"k8_{g}", name=f"k8_{g}") for g in range(4)]
        Q8 = [pbig.tile([128, 2, NQ], F8, tag=f"q8_{g}", name=f"q8_{g}") for g in range(4)]
        # V for the AV matmul: [token-part, kt, head, 65]; col 64 = ones
        VO = pbig.tile([128, KT, H, 65], BF16, tag="vo")
        # attention output transposed: AT[hp] rows = heads 2hp,2hp+1 (64 d each)
        AT = [pbig.tile([128, NQ], BF16, tag=f"at{p}", name=f"at{p}") for p in range(H // 2)]

        nc.vector.memset(VO[:, :, :, 64:65], 1.0)

        # biases: bqkvT[p, c] = bqkv[128c + p] via strided DRAM read
        bqkvT = pc.tile([128, 24], F32, tag="bqkvT")
        nc.sync.dma_start(out=bqkvT, in_=bqkv.rearrange("(c p) -> p c", p=128))
        boT = pc.tile([128, E], BF16, tag="boT")
        bo_bcast = bass.AP(tensor=bo.tensor, offset=bo.offset, ap=[[0, 128]] + bo.ap)
        nc.gpsimd.dma_start(out=boT, in_=bo_bcast)
        boB = pc.tile([128, E], BF16, tag="boB")

        # --- Q/K projection chains (fp8 DoubleRow, contraction 256/step) -
        def kq_chain(kind, t, tb, stage):
            # one K^T/Q^T proj tile [128, 512 tokens] for pair-tile t,
            # written (descaled) with bias into the fp8 stage tile.
            wc0 = (E if kind == "k" else 0) + t * 128
            bcol = wc0 // 128
            ps = ppj.tile([128, 512], F32, tag="ps")
            for a in range(4):
                nc.tensor.matmul(
                    ps,
                    lhsT=w8[:, a, :, wc0 : wc0 + 128],
                    rhs=xT8[:, 2 * a : 2 * a + 2, tb * 512 : (tb + 1) * 512],
                    start=(a == 0),
                    stop=(a == 3),
                    perf_mode=DR,
                )
            nc.vector.tensor_scalar(
                out=stage[:, tb * 512 : (tb + 1) * 512], in0=ps,
                scalar1=1.0 / WSCALE, scalar2=bqkvT[:, bcol : bcol + 1],
                op0=ALU.mult, op1=ALU.add,
            )

        def kq_folds(kind, t, stage, tb=None):
            # stage [128, ntok] fp8 -> K8/Q8[g] partition fold (4 DMAs);
            # tb=None folds the whole token range, else just that 512-block
            g = t // 2
            dst = K8[g] if kind == "k" else Q8[g]
            a0 = 2 * (t % 2)
            c0, c1 = (0, dst.shape[-1]) if tb is None else (tb * 512, (tb + 1) * 512)
            for s in range(4):
                nc.sync.dma_start(
                    out=dst[32 * (a0 + s // 2) : 32 * (a0 + s // 2) + 32, s % 2, c0:c1],
                    in_=stage[32 * s : 32 * s + 32, c0:c1],
                )

        def v_quarter(tt, qq):
            # bf16 V proj for token tile tt, heads 4qq..4qq+3 (256 w-cols)
            ps = ppj.tile([128, 512], F32, tag="ps")
            wc0 = qq * 256
            for ee in range(8):
                nc.tensor.matmul(
                    ps[:, 0:256],
                    lhsT=xTb[:, ee, tt * 128 : (tt + 1) * 128],
                    rhs=wvbf[:, ee, wc0 : wc0 + 256],
                    start=(ee == 0),
                    stop=(ee == 7),
                )
            nc.vector.tensor_copy(
                VO[:, tt, 4 * qq : 4 * qq + 4, 0:64],
                ps[:, 0:256].rearrange("p (h d) -> p h d", d=64),
            )

        def v_half(tt, vh):
            # bf16 V proj for token tile tt, heads 8vh..8vh+7 (512 w-cols):
            # half the instruction count of two quarters, used once the
            # window schedule has room for the bigger jobs.
            ps = ppj.tile([128, 512], F32, tag="ps")
            wc0 = vh * 512
            for ee in range(8):
                nc.tensor.matmul(
                    ps,
                    lhsT=xTb[:, ee, tt * 128 : (tt + 1) * 128],
                    rhs=wvbf[:, ee, wc0 : wc0 + 512],
                    start=(ee == 0),
                    stop=(ee == 7),
                )
            nc.vector.tensor_copy(
                VO[:, tt, 8 * vh : 8 * vh + 8, 0:64],
                ps.rearrange("p (h d) -> p h d", d=64),
            )

        # --- out-proj ---------------------------------------------------
        def wob(half):
            return wobf_box[0][:, :, half * 512 : (half + 1) * 512]

        def boB_setup():
            # attn-out = AV/den + bv  (V-bias passes softmax unchanged), so
            # out = A_nobias @ Wo + (bv @ Wo + bo) = A_nobias @ Wo + boB
            ones128 = pc.tile([128, 128], BF16, tag="ones128")
            nc.vector.memset(ones128, 1.0)
            bv_rep = pc.tile([128, 8, 128], BF16, tag="bvrep")
            for ee in range(8):
                nc.vector.tensor_scalar_mul(
                    bv_rep[:, ee, :], ones128, bqkvT[:, 16 + ee : 17 + ee]
                )
            for half in range(2):
                c0 = half * 512
                psb = ppj.tile([128, 512], F32, tag="ps")
                for ee in range(8):
                    nc.tensor.matmul(
                        psb,
                        lhsT=bv_rep[:, ee, :],
                        rhs=wob(half)[:, ee, :],
                        start=(ee == 0),
                        stop=(ee == 7),
                    )
                nc.vector.tensor_add(boB[:, c0 : c0 + 512], psb, boT[:, c0 : c0 + 512])

        # three-stage out-proj: pass A (e-chunks 0:6 = pairs 0-5, windows
        # 12-14) -> bf16 partials incl. boB; pass B1 (chunk 6, windows
        # 14-15) adds pair 6; pass B2 (chunk 7) is the only tail work.
        NEA = 6
        partial = pprt.tile([128, 16, 512], BF16, tag="partial")

        def outproj_passA(tt, half):
            c0 = half * 512
            ps = ppj.tile([128, 512], F32, tag="ps")
            for ee in range(NEA):
                nc.tensor.matmul(
                    ps,
                    lhsT=AT[ee][:, tt * 128 : (tt + 1) * 128],
                    rhs=wob(half)[:, ee, :],
                    start=(ee == 0),
                    stop=(ee == NEA - 1),
                )
            nc.vector.tensor_add(
                partial[:, tt * 2 + half, :], ps, boB[:, c0 : c0 + 512]
            )

        def outproj_passB2(tt):
            # covers e-chunk pairs 6 and 7 in one accumulated pass
            ps = pps.tile([128, NQ], F32, tag="ps_s", name=f"psb2_{tt}")
            for half in range(2):
                for ee in (NEA, 7):
                    nc.tensor.matmul(
                        ps[:, half * 512 : (half + 1) * 512],
                        lhsT=AT[ee][:, tt * 128 : (tt + 1) * 128],
                        rhs=wob(half)[:, ee, :],
                        start=(ee == NEA),
                        stop=(ee == 7),
                    )
            osb = posb_box[0].tile([128, E], F32, tag="osb", name=f"osb{tt}")
            nc.vector.tensor_add(
                osb, ps, partial.rearrange("p t c -> p (t c)")[:, tt * 1024 : (tt + 1) * 1024]
            )
            nc.sync.dma_start(out=out[tt * 128 : (tt + 1) * 128, :], in_=osb)

        # --- prologue ---------------------------------------------------
        # PE warmup: keep the tensor engine busy from t~1us so the p-state
        # ramp completes before the first real chain arrives.
        warm = pc.tile([128, 256], F32, tag="warm")
        nc.vector.memset(warm, 0.001)
        ps_w = pps.tile([128, NQ], F32, tag="ps_s", name="warmps")
        for i in range(5):
            nc.tensor.matmul(
                ps_w[:, 0:256], lhsT=warm[:, 0:128], rhs=warm,
                start=True, stop=True, skip_group_check=True,
            )

        # x + weights: DMAs chunked ~0.5-1MB and ordered so the t=0 chain
        # inputs land first; the shared DMA bandwidth drains the FIFO in
        # ready-order, so anything big in front delays the tiny stage fold
        # copies that gate the first scores. The fp8 x copy is produced
        # on-chip (DVE for the first chunks, GPSIMD for the rest - both
        # near-idle here) instead of shipping a second 2MB copy of x.
        nc.sync.dma_start(out=w8[:, :, :, 0:512], in_=w8d[:, :, :, 0:512])
        for tk in range(8):
            t0, t1 = 256 * tk, 256 * tk + 256
            eng = (nc.scalar, nc.sync)[tk % 2]
            eng.dma_start(out=xTb[:, :, t0:t1], in_=xTbd[:, :, t0:t1])
            conv = nc.vector if tk in (0, 1, 2, 3, 6, 7) else nc.gpsimd
            conv.tensor_copy(xT8[:, :, t0:t1], xTb[:, :, t0:t1])
            if tk == 1:
                nc.scalar.dma_start(
                    out=wvbf[:, :, 0:256], in_=wvbfd[:, :, 0:256]
                )
            elif tk == 2:
                nc.sync.dma_start(
                    out=w8[:, :, :, E : E + 512], in_=w8d[:, :, :, E : E + 512]
                )
            elif tk == 4:
                nc.scalar.dma_start(
                    out=w8[:, :, :, E + 512 : 2 * E], in_=w8d[:, :, :, E + 512 : 2 * E]
                )
            elif tk == 5:
                nc.sync.dma_start(out=w8[:, :, :, 512:E], in_=w8d[:, :, :, 512:E])

        stg_q0 = pqstg.tile([128, NQ], F8, tag="qstg", name="qstg0")
        stg_k0 = pkstg.tile([128, SEQ], F8, tag="kstg", name="kstg0")
        for tb in range(2):
            kq_chain("q", 0, tb, stg_q0)
            kq_folds("q", 0, stg_q0, tb=tb)
        for tb in range(4):
            kq_chain("k", 0, tb, stg_k0)
            kq_folds("k", 0, stg_k0, tb=tb)

        # remaining wvbf quarters (heads 4-15) land behind the fold copies
        for qq in range(1, 4):
            eng = (nc.sync, nc.scalar)[qq % 2]
            eng.dma_start(
                out=wvbf[:, :, 256 * qq : 256 * qq + 256],
                in_=wvbfd[:, :, 256 * qq : 256 * qq + 256],
            )

        # --- deferred proj work, paced one job per kt slot ---------------
        def kq_jobs(kind, t):
            # one chain per job; folds ride with the last chain
            ntb = 4 if kind == "k" else 2
            stage_box = {}

            def mk(tb):
                def job():
                    if tb == 0:
                        pool, shape = (pkstg, SEQ) if kind == "k" else (pqstg, NQ)
                        stage_box[0] = pool.tile(
                            [128, shape], F8, tag="kstg" if kind == "k" else "qstg",
                            name=f"{kind}stg{t}",
                        )
                    kq_chain(kind, t, tb, stage_box[0])
                    if tb == ntb - 1:
                        kq_folds(kind, t, stage_box[0])

                return job

            return [mk(tb) for tb in range(ntb)]

        def vq(qq, ts):
            return [(lambda t=t: v_quarter(t, qq)) for t in ts]

        def vh2(ts):
            return [(lambda t=t: v_half(t, 1)) for t in ts]

        def load_wobf():
            wobf_box[0] = plate_box[0].tile([128, 8, E], BF16, tag="wobf", name="wobf")
            nc.sync.dma_start(out=wobf_box[0], in_=wobfd)

        plate_box = {}
        posb_box = {}
        def passA_jobs(ts, half):
            return [(lambda t=t: outproj_passA(t, half)) for t in ts]

        jobs = {
            0: vq(0, range(16)),
            1: kq_jobs("k", 1) + kq_jobs("q", 1) + vq(1, range(0, 2)),
            2: vq(1, range(2, 10)),
            3: kq_jobs("k", 2) + kq_jobs("q", 2) + vq(1, range(10, 16)),
            4: kq_jobs("k", 3) + kq_jobs("q", 3),
            5: vh2(range(0, 6)),
            6: vh2(range(6, 11)) + kq_jobs("q", 4),
            7: kq_jobs("k", 4) + kq_jobs("q", 5) + vh2(range(11, 14)),
            8: kq_jobs("k", 5) + kq_jobs("q", 6) + vh2(range(14, 16)),
            9: kq_jobs("k", 6) + kq_jobs("q", 7),
            10: kq_jobs("k", 7),
            11: [load_wobf],
            12: [boB_setup] + passA_jobs(range(3), 0),
            13: passA_jobs(range(3), 1) + passA_jobs(range(3, 6), 0),
            14: passA_jobs(range(3, 6), 1) + passA_jobs(range(6, 8), 0),
            15: passA_jobs(range(6, 8), 1),
        }

        # --- attention --------------------------------------------------
        cur_asb = [None] * QT
        pending_norm = [[]]

        def attention_head(h, inner=None):
            g, a = h // 4, h % 4
            p0 = 32 * a
            av = [
                pav.tile([128, 4, 65], F32, tag="av", name=f"av{h}_{i}")
                for i in range(2)
            ]

            def av_half(pt, kt, hv):
                for q4 in range(4):
                    qt = 4 * hv + q4
                    nc.tensor.matmul(
                        av[hv][:, q4, :],
                        lhsT=pt[:, qt * 128 : (qt + 1) * 128],
                        rhs=VO[:, kt, h, :],
                        start=(kt == 0 and q4 == 0),
                        stop=(kt == KT - 1),
                        skip_group_check=True,
                    )

            # AV for tile kt is emitted 1 slot late (av half 0) / 2 slots
            # late (half 1): the exp-wait then never head-of-line blocks the
            # next scores, and the head-boundary av-tile reuse (pav bufs=2)
            # has time to clear through the deferred normalize jobs.
            hist = []
            for kt in range(KT):
                ps_s = pps.tile([128, NQ], F32, tag="ps_s")
                for qh in range(2):
                    nc.tensor.matmul(
                        ps_s[:, qh * 512 : (qh + 1) * 512],
                        lhsT=K8[g][p0 : p0 + 32, :, kt * 128 : (kt + 1) * 128],
                        rhs=Q8[g][p0 : p0 + 32, :, qh * 512 : (qh + 1) * 512],
                        start=True,
                        stop=True,
                        perf_mode=DR,
                        tile_position=(p0, 0),
                    )
                pt = ppt.tile([128, NQ], BF16, tag="pt")
                # head 0: DVE is busy producing the fp8 x copy; keep its
                # whole exp stream on the scalar engine
                if kt in DVE_KT and h != 0:
                    nc.vector.tensor_scalar(
                        out=pt.bitcast(I16), in0=ps_s,
                        scalar1=S16_MUL, scalar2=S16_ADD,
                        op0=ALU.mult, op1=ALU.add,
                    )
                else:
                    nc.scalar.activation(pt, ps_s, AF.Exp, scale=0.125)
                if len(hist) >= 1:
                    av_half(hist[-1][0], hist[-1][1], 0)
                if len(hist) >= 2:
                    av_half(hist[-2][0], hist[-2][1], 1)
                if inner is not None and kt < len(inner):
                    inner[kt]()
                hist = hist[-1:] + [(pt, kt)]
            av_half(hist[-2][0], hist[-2][1], 1)
            av_half(hist[-1][0], hist[-1][1], 0)
            av_half(hist[-1][0], hist[-1][1], 1)

            # normalize (reciprocal + scale + AT transpose) is deferred into
            # the next head's first slots so it never stalls the exp stream.
            hp = h // 2
            rec_box = {}

            def mk_norm(qt):
                def job():
                    if qt % 4 == 0:
                        # one reciprocal covers the 4 denominators of av[hv]
                        rec_box[qt // 4] = prec.tile(
                            [128, 4, 1], F32, tag="rec", name=f"rec{h}_{qt // 4}"
                        )
                        nc.vector.reciprocal_approx_fast(
                            rec_box[qt // 4], av[qt // 4][:, :, 64:65]
                        )
                    if h % 2 == 0:
                        cur_asb[qt] = pasb.tile(
                            [128, 128], BF16, tag="asb", name=f"asb{h}_{qt}"
                        )
                    nc.vector.tensor_scalar_mul(
                        cur_asb[qt][:, 64 * (h % 2) : 64 * (h % 2) + 64],
                        av[qt // 4][:, qt % 4, 0:64],
                        rec_box[qt // 4][:, qt % 4, :],
                    )
                    if h % 2 == 1:
                        nc.sync.dma_start(
                            out=AT[hp][:, qt * 128 : (qt + 1) * 128],
                            in_=cur_asb[qt],
                            transpose=True,
                        )

                return job

            pending_norm[0] = [mk_norm(qt) for qt in range(QT)]

        for h in range(H):
            if h == 11:
                # all Q/K chains are emitted; reuse xT8/w8 space for wobf
                # and the out-proj staging buffers
                pearly_es.close()
                plate_box[0] = es.enter_context(tc.tile_pool(name="late", bufs=1))
                posb_box[0] = es.enter_context(tc.tile_pool(name="osb", bufs=4))
            norms = pending_norm[0]
            pending_norm[0] = []
            norm_jobs = [
                (lambda grp=norms[i : i + 4]: [j() for j in grp])
                for i in range(0, len(norms), 4)
            ]
            window_jobs = jobs.get(h, [])
            inner = norm_jobs + [(lambda j=j: j()) for j in window_jobs]
            attention_head(h, inner=inner)

        # --- tail: last head's normalize, then out-proj pass B2 ----------
        for j in pending_norm[0]:
            j()
        for tt in range(8):
            outproj_passB2(tt)


_NC = None


def _get_program():
    global _NC
    if _NC is None:
        _NC = build_program()
    return _NC


def make_in_maps(x, Wqkv, bqkv, Wo, bo):
    f8 = np.dtype(mybir.dt.np(F8))
    bf16 = np.dtype(mybir.dt.np(BF16))
    Wqkv = np.asarray(Wqkv, np.float32)
    w8 = (Wqkv[:, : 2 * E] * WSCALE).reshape(4, 2, 128, 2 * E)
    w8 = np.ascontiguousarray(w8.transpose(2, 0, 1, 3)).astype(f8)
    wvbf = Wqkv[:, 2 * E :].reshape(8, 128, E)
    wvbf = np.ascontiguousarray(wvbf.transpose(1, 0, 2)).astype(bf16)
    wobf = np.asarray(Wo, np.float32).reshape(8, 128, E)
    wobf = np.ascontiguousarray(wobf.transpose(1, 0, 2)).astype(bf16)
    w = {
        "w8": w8,
        "wvbf": wvbf,
        "wobf": wobf,
        "bqkv": np.ascontiguousarray(np.asarray(bqkv, np.float32)),
        "bo": np.ascontiguousarray(np.asarray(bo, np.float32)),
    }
    x = np.asarray(x, np.float32)
    in_maps = []
    for c in range(N_CORES):
        b, s = divmod(c, 2)
        xb = x[b]
        if s == 1:
            xb = np.roll(xb, -NQ, axis=0)
        xTb = np.ascontiguousarray(
            xb.T.reshape(8, 128, SEQ).transpose(1, 0, 2)
        ).astype(bf16)
        in_maps.append({"xTb": xTb, **w})
    return in_maps


def gather_out(results):
    out = np.empty((4, SEQ, E), np.float32)
    for c in range(N_CORES):
        b, s = divmod(c, 2)
        out[b, s * NQ : (s + 1) * NQ] = results[c]["out"]
    return out


def kernel(x, Wqkv, bqkv, Wo, bo):
    from concourse.bass_utils import run_bass_kernel_spmd

    nc = _get_program()
    in_maps = make_in_maps(x, Wqkv, bqkv, Wo, bo)
    res = run_bass_kernel_spmd(nc, in_maps, core_ids=list(range(N_CORES)))
    return gather_out(res.results)


# revision 64
# speedup vs baseline: 1.0124x; 1.0124x over previous
"""Trainium2 Bass kernel for a 16-head MHA layer (batch 4, seq 2048, embed 1024).

Sharding: 8 cores; core c handles batch c//2 and query-token half c%2.
Each core receives its batch's x rotated so that its 1024 query tokens sit in
rows 0:1024 (softmax/attention is permutation-invariant over key order, so the
rotation changes nothing mathematically). K/V are computed over the full
sequence on-core, so no collectives are needed. Weights are replicated.

Host-side prep (layout/dtype only): x is passed transposed twice - bf16
panels (xTb[p, ee, t] = x[t, 128*ee+p]) for the V projection, and an
fp8 e-dim-folded copy (xT8[p, a, h, t] = x[t, 256a+128h+p]) for the Q/K
projections; W_qk is fp8 pre-scaled by 256 in the matching fold; W_v and
W_o are bf16 panels. All fp8 is e4m3.

Compute structure per core:
 - Q/K projections as fp8 DoubleRow matmuls (contraction 256/step); the
   1/256 weight prescale is undone in the fused stage op (PSUM -> fp8
   stage with scale+bias on DVE). V projection stays bf16: V-side noise
   passes straight into the output, while Q/K noise only perturbs
   softmax weights, so fp8 is affordable only for Q/K.
 - Scores run as fp8 DoubleRow matmuls over the head dim (d=64 folded as
   32 partitions x 2), K stationary, 512 q per step.
 - exp() is split across two engines: most tiles on the scalar engine,
   a subset on DVE via an integer Schraudolph trick: n = round(a*s + b)
   written as int16 and bit-reinterpreted as bf16 approximates exp(s/8)
   to ~1.8% rms. Softmax normalizes away shared scale, so only relative
   weight noise matters.
 - AV is bf16; P^T is the stationary operand and V streams 64+1 columns
   (the ones-column produces the softmax denominator in the same
   matmul). AV for key-tile kt is emitted one slot late (after the
   scores of kt+1) so its wait on exp(kt) never head-of-line blocks the
   next scores in the in-order PE queue.
 - Per (head, q-tile): reciprocal of the denominator + fused scale on DVE,
   then a DMA-transpose flips the pair back to [d, q] for the out-proj.
 - V-bias folds through softmax (sum(attn)=1) into the out-proj bias.
"""

import sys

for _p in ("/opt/trn_rl_repo",):
    if _p not in sys.path:
        sys.path.insert(0, _p)

import numpy as np

import concourse.bass as bass  # noqa: E402
import concourse.mybir as mybir  # noqa: E402
import concourse.tile as tile  # noqa: E402
from concourse import bacc  # noqa: E402

SEQ = 2048
E = 1024
H = 16
D = 64
NQ = 1024  # query tokens per core
N_CORES = 8

TT = SEQ // 128  # 16 token tiles
KT = SEQ // 128  # 16 key tiles
QT = NQ // 128   # 8 query tiles

F32 = mybir.dt.float32
BF16 = mybir.dt.bfloat16
F8 = mybir.dt.float8e4
I16 = mybir.dt.int16
AF = mybir.ActivationFunctionType
DR = mybir.MatmulPerfMode.DoubleRow
ALU = mybir.AluOpType

WSCALE = 256.0               # W_qk prescale (keeps fp8 weights out of denormals)
S16_MUL = 23.083120654223414  # 128 * log2(e) * 0.125
S16_ADD = 16248.66            # 128*(127 - E[log2(1+t)-t]): mean-unbiased
DVE_KT = (5, 7, 9, 11, 13)    # key tiles whose exp runs on DVE (rest: scalar)


def build_program():
    nc = bacc.Bacc(trn_type="TRN2", target_bir_lowering=False, debug=False)

    xTb = nc.dram_tensor("xTb", [128, 8, SEQ], BF16, kind="ExternalInput").ap()
    w8 = nc.dram_tensor("w8", [128, 4, 2, 2 * E], F8, kind="ExternalInput").ap()
    wvbf = nc.dram_tensor("wvbf", [128, 8, E], BF16, kind="ExternalInput").ap()
    wobf = nc.dram_tensor("wobf", [128, 8, E], BF16, kind="ExternalInput").ap()
    bqkv = nc.dram_tensor("bqkv", [3 * E], F32, kind="ExternalInput").ap()
    bo = nc.dram_tensor("bo", [E], F32, kind="ExternalInput").ap()
    out = nc.dram_tensor("out", [NQ, E], F32, kind="ExternalOutput").ap()

    with tile.TileContext(nc) as tc:
        _body(nc, tc, xTb, w8, wvbf, wobf, bqkv, bo, out)

    nc.compile()
    return nc


def _body(nc, tc, xTbd, w8d, wvbfd, wobfd, bqkv, bo, out):
    from contextlib import ExitStack

    es = ExitStack()
    with es:
        pc = es.enter_context(tc.tile_pool(name="const", bufs=1))
        pbig = es.enter_context(tc.tile_pool(name="big", bufs=1))
        pprt = es.enter_context(tc.tile_pool(name="part", bufs=1))
        pkstg = es.enter_context(tc.tile_pool(name="kstg", bufs=1))
        pqstg = es.enter_context(tc.tile_pool(name="qstg", bufs=1))
        ppt = es.enter_context(tc.tile_pool(name="pt", bufs=4))
        pasb = es.enter_context(tc.tile_pool(name="asb", bufs=10))
        prec = es.enter_context(tc.tile_pool(name="rec", bufs=4))
        pps = es.enter_context(tc.tile_pool(name="ps_s", bufs=2, space="PSUM"))
        ppj = es.enter_context(tc.tile_pool(name="ps_pj", bufs=2, space="PSUM"))
        pav = es.enter_context(tc.tile_pool(name="ps_av", bufs=2, space="PSUM"))

        # xT8/w8 die after the last Q/K chain (window 10); wobf reuses
        # their space (the "late" pool opens after pearly closes).
        pearly_es = ExitStack()
        pearly = pearly_es.enter_context(tc.tile_pool(name="early", bufs=1))

        # --- persistent SBUF tensors -----------------------------------
        # xT8[p, ee, t] = fp8(xTb[p, ee, t]): the e-dim fold for DoubleRow is
        # (p, ee parity), so the fp8 copy shares xTb's layout exactly.
        xT8 = pearly.tile([128, 8, SEQ], F8, tag="xT8")
        w8 = pearly.tile([128, 4, 2, 2 * E], F8, tag="w8")
        xTb = pbig.tile([128, 8, SEQ], BF16, tag="xTb")
        wvbf = pbig.tile([128, 8, E], BF16, tag="wvbf")
        wobf_box = {}
        # fp8 DoubleRow K/Q: tile g holds heads 4g..4g+3 at partition blocks
        # 32a; free dims = [d-half, token]
        K8 = [pbig.tile([128, 2, SEQ], F8, tag=f"k8_{g}", name=f"k8_{g}") for g in range(4)]
        Q8 = [pbig.tile([128, 2, NQ], F8, tag=f"q8_{g}", name=f"q8_{g}") for g in range(4)]
        # V for the AV matmul: [token-part, kt, head, 65]; col 64 = ones
        VO = pbig.tile([128, KT, H, 65], BF16, tag="vo")
        # attention output transposed: AT[hp] rows = heads 2hp,2hp+1 (64 d each)
        AT = [pbig.tile([128, NQ], BF16, tag=f"at{p}", name=f"at{p}") for p in range(H // 2)]

        nc.vector.memset(VO[:, :, :, 64:65], 1.0)

        # biases: bqkvT[p, c] = bqkv[128c + p] via strided DRAM read
        bqkvT = pc.tile([128, 24], F32, tag="bqkvT")
        nc.sync.dma_start(out=bqkvT, in_=bqkv.rearrange("(c p) -> p c", p=128))
        boT = pc.tile([128, E], BF16, tag="boT")
        bo_bcast = bass.AP(tensor=bo.tensor, offset=bo.offset, ap=[[0, 128]] + bo.ap)
        nc.gpsimd.dma_start(out=boT, in_=bo_bcast)
        boB = pc.tile([128, E], BF16, tag="boB")

        # --- Q/K projection chains (fp8 DoubleRow, contraction 256/step) -
        def kq_chain(kind, t, tb, stage):
            # one K^T/Q^T proj tile [128, 512 tokens] for pair-tile t,
            # written (descaled) with bias into the fp8 stage tile.
            wc0 = (E if kind == "k" else 0) + t * 128
            bcol = wc0 // 128
            ps = ppj.tile([128, 512], F32, tag="ps")
            for a in range(4):
                nc.tensor.matmul(
                    ps,
                    lhsT=w8[:, a, :, wc0 : wc0 + 128],
                    rhs=xT8[:, 2 * a : 2 * a + 2, tb * 512 : (tb + 1) * 512],
                    start=(a == 0),
                    stop=(a == 3),
                    perf_mode=DR,
                )
            nc.vector.tensor_scalar(
                out=stage[:, tb * 512 : (tb + 1) * 512], in0=ps,
                scalar1=1.0 / WSCALE, scalar2=bqkvT[:, bcol : bcol + 1],
                op0=ALU.mult, op1=ALU.add,
            )

        def kq_folds(kind, t, stage, tb=None):
            # stage [128, ntok] fp8 -> K8/Q8[g] partition fold (4 DMAs);
            # tb=None folds the whole token range, else just that 512-block
            g = t // 2
            dst = K8[g] if kind == "k" else Q8[g]
            a0 = 2 * (t % 2)
            c0, c1 = (0, dst.shape[-1]) if tb is None else (tb * 512, (tb + 1) * 512)
            for s in range(4):
                nc.sync.dma_start(
                    out=dst[32 * (a0 + s // 2) : 32 * (a0 + s // 2) + 32, s % 2, c0:c1],
                    in_=stage[32 * s : 32 * s + 32, c0:c1],
                )

        def v_quarter(tt, qq):
            # bf16 V proj for token tile tt, heads 4qq..4qq+3 (256 w-cols)
            ps = ppj.tile([128, 512], F32, tag="ps")
            wc0 = qq * 256
            for ee in range(8):
                nc.tensor.matmul(
                    ps[:, 0:256],
                    lhsT=xTb[:, ee, tt * 128 : (tt + 1) * 128],
                    rhs=wvbf[:, ee, wc0 : wc0 + 256],
                    start=(ee == 0),
                    stop=(ee == 7),
                )
            nc.vector.tensor_copy(
                VO[:, tt, 4 * qq : 4 * qq + 4, 0:64],
                ps[:, 0:256].rearrange("p (h d) -> p h d", d=64),
            )

        def v_half(tt, vh):
            # bf16 V proj for token tile tt, heads 8vh..8vh+7 (512 w-cols):
            # half the instruction count of two quarters, used once the
            # window schedule has room for the bigger jobs.
            ps = ppj.tile([128, 512], F32, tag="ps")
            wc0 = vh * 512
            for ee in range(8):
                nc.tensor.matmul(
                    ps,
                    lhsT=xTb[:, ee, tt * 128 : (tt + 1) * 128],
                    rhs=wvbf[:, ee, wc0 : wc0 + 512],
                    start=(ee == 0),
                    stop=(ee == 7),
                )
            nc.vector.tensor_copy(
                VO[:, tt, 8 * vh : 8 * vh + 8, 0:64],
                ps.rearrange("p (h d) -> p h d", d=64),
            )

        # --- out-proj ---------------------------------------------------
        def wob(half):
            return wobf_box[0][:, :, half * 512 : (half + 1) * 512]

        def boB_setup():
            # attn-out = AV/den + bv  (V-bias passes softmax unchanged), so
            # out = A_nobias @ Wo + (bv @ Wo + bo) = A_nobias @ Wo + boB
            ones128 = pc.tile([128, 128], BF16, tag="ones128")
            nc.vector.memset(ones128, 1.0)
            bv_rep = pc.tile([128, 8, 128], BF16, tag="bvrep")
            for ee in range(8):
                nc.vector.tensor_scalar_mul(
                    bv_rep[:, ee, :], ones128, bqkvT[:, 16 + ee : 17 + ee]
                )
            for half in range(2):
                c0 = half * 512
                psb = ppj.tile([128, 512], F32, tag="ps")
                for ee in range(8):
                    nc.tensor.matmul(
                        psb,
                        lhsT=bv_rep[:, ee, :],
                        rhs=wob(half)[:, ee, :],
                        start=(ee == 0),
                        stop=(ee == 7),
                    )
                nc.vector.tensor_add(boB[:, c0 : c0 + 512], psb, boT[:, c0 : c0 + 512])

        # three-stage out-proj: pass A (e-chunks 0:6 = pairs 0-5, windows
        # 12-14) -> bf16 partials incl. boB; pass B1 (chunk 6, windows
        # 14-15) adds pair 6; pass B2 (chunk 7) is the only tail work.
        NEA = 6
        partial = pprt.tile([128, 16, 512], BF16, tag="partial")

        def outproj_passA(tt, half):
            c0 = half * 512
            ps = ppj.tile([128, 512], F32, tag="ps")
            for ee in range(NEA):
                nc.tensor.matmul(
                    ps,
                    lhsT=AT[ee][:, tt * 128 : (tt + 1) * 128],
                    rhs=wob(half)[:, ee, :],
                    start=(ee == 0),
                    stop=(ee == NEA - 1),
                )
            nc.vector.tensor_add(
                partial[:, tt * 2 + half, :], ps, boB[:, c0 : c0 + 512]
            )

        def outproj_passB2(tt):
            # covers e-chunk pairs 6 and 7 in one accumulated pass
            ps = pps.tile([128, NQ], F32, tag="ps_s", name=f"psb2_{tt}")
            for half in range(2):
                for ee in (NEA, 7):
                    nc.tensor.matmul(
                        ps[:, half * 512 : (half + 1) * 512],
                        lhsT=AT[ee][:, tt * 128 : (tt + 1) * 128],
                        rhs=wob(half)[:, ee, :],
                        start=(ee == NEA),
                        stop=(ee == 7),
                    )
            osb = posb_box[0].tile([128, E], F32, tag="osb", name=f"osb{tt}")
            nc.vector.tensor_add(
                osb, ps, partial.rearrange("p t c -> p (t c)")[:, tt * 1024 : (tt + 1) * 1024]
            )
            nc.sync.dma_start(out=out[tt * 128 : (tt + 1) * 128, :], in_=osb)

        # --- prologue ---------------------------------------------------
        # PE warmup: keep the tensor engine busy from t~1us so the p-state
        # ramp completes before the first real chain arrives.
        warm = pc.tile([128, 256], F32, tag="warm")
        nc.vector.memset(warm, 0.001)
        ps_w = pps.tile([128, NQ], F32, tag="ps_s", name="warmps")
        for i in range(5):
            nc.tensor.matmul(
                ps_w[:, 0:256], lhsT=warm[:, 0:128], rhs=warm,
                start=True, stop=True, skip_group_check=True,
            )

        # x + weights: DMAs chunked ~0.5-1MB and ordered so the t=0 chain
        # inputs land first; the shared DMA bandwidth drains the FIFO in
        # ready-order, so anything big in front delays the tiny stage fold
        # copies that gate the first scores. The fp8 x copy is produced
        # on-chip (DVE for the first chunks, GPSIMD for the rest - both
        # near-idle here) instead of shipping a second 2MB copy of x.
        nc.sync.dma_start(out=w8[:, :, :, 0:512], in_=w8d[:, :, :, 0:512])
        for tk in range(8):
            t0, t1 = 256 * tk, 256 * tk + 256
            eng = (nc.scalar, nc.sync)[tk % 2]
            eng.dma_start(out=xTb[:, :, t0:t1], in_=xTbd[:, :, t0:t1])
            conv = nc.vector if tk in (0, 1, 2, 3, 6, 7) else nc.gpsimd
            conv.tensor_copy(xT8[:, :, t0:t1], xTb[:, :, t0:t1])
            if tk == 1:
                nc.scalar.dma_start(
                    out=wvbf[:, :, 0:256], in_=wvbfd[:, :, 0:256]
                )
            elif tk == 2:
                nc.sync.dma_start(
                    out=w8[:, :, :, E : E + 512], in_=w8d[:, :, :, E : E + 512]
                )
            elif tk == 4:
                nc.scalar.dma_start(
                    out=w8[:, :, :, E + 512 : 2 * E], in_=w8d[:, :, :, E + 512 : 2 * E]
                )
            elif tk == 5:
                nc.sync.dma_start(out=w8[:, :, :, 512:E], in_=w8d[:, :, :, 512:E])

        stg_q0 = pqstg.tile([128, NQ], F8, tag="qstg", name="qstg0")
        stg_k0 = pkstg.tile([128, SEQ], F8, tag="kstg", name="kstg0")
        for tb in range(2):
            kq_chain("q", 0, tb, stg_q0)
            kq_folds("q", 0, stg_q0, tb=tb)
        for tb in range(4):
            kq_chain("k", 0, tb, stg_k0)
            kq_folds("k", 0, stg_k0, tb=tb)

        # remaining wvbf quarters (heads 4-15) land behind the fold copies
        for qq in range(1, 4):
            eng = (nc.sync, nc.scalar)[qq % 2]
            eng.dma_start(
                out=wvbf[:, :, 256 * qq : 256 * qq + 256],
                in_=wvbfd[:, :, 256 * qq : 256 * qq + 256],
            )

        # --- deferred proj work, paced one job per kt slot ---------------
        def kq_jobs(kind, t):
            # one chain per job; folds ride with the last chain
            ntb = 4 if kind == "k" else 2
            stage_box = {}

            def mk(tb):
                def job():
                    if tb == 0:
                        pool, shape = (pkstg, SEQ) if kind == "k" else (pqstg, NQ)
                        stage_box[0] = pool.tile(
                            [128, shape], F8, tag="kstg" if kind == "k" else "qstg",
                            name=f"{kind}stg{t}",
                        )
                    kq_chain(kind, t, tb, stage_box[0])
                    if tb == ntb - 1:
                        kq_folds(kind, t, stage_box[0])

                return job

            return [mk(tb) for tb in range(ntb)]

        def vq(qq, ts):
            return [(lambda t=t: v_quarter(t, qq)) for t in ts]

        def vh2(ts):
            return [(lambda t=t: v_half(t, 1)) for t in ts]

        def load_wobf():
            wobf_box[0] = plate_box[0].tile([128, 8, E], BF16, tag="wobf", name="wobf")
            nc.sync.dma_start(out=wobf_box[0], in_=wobfd)

        plate_box = {}
        posb_box = {}
        def passA_jobs(ts, half):
            return [(lambda t=t: outproj_passA(t, half)) for t in ts]

        jobs = {
            0: vq(0, range(16)),
            1: kq_jobs("k", 1) + kq_jobs("q", 1) + vq(1, range(0, 2)),
            2: vq(1, range(2, 10)),
            3: kq_jobs("k", 2) + kq_jobs("q", 2) + vq(1, range(10, 16)),
            4: kq_jobs("k", 3) + kq_jobs("q", 3),
            5: vq(2, range(0, 10)),
            6: vq(2, range(10, 16)) + kq_jobs("q", 4),
            7: kq_jobs("k", 4) + kq_jobs("q", 5) + vq(3, range(0, 2)),
            8: kq_jobs("k", 5) + kq_jobs("q", 6) + vq(3, range(2, 4)),
            9: kq_jobs("k", 6) + kq_jobs("q", 7) + vq(3, range(4, 6)),
            10: kq_jobs("k", 7) + vq(3, range(6, 12)),
            11: [load_wobf] + vq(3, range(12, 16)),
            12: [boB_setup] + passA_jobs(range(3), 0),
            13: passA_jobs(range(3), 1) + passA_jobs(range(3, 6), 0),
            14: passA_jobs(range(3, 6), 1) + passA_jobs(range(6, 8), 0),
            15: passA_jobs(range(6, 8), 1),
        }

        # --- attention --------------------------------------------------
        cur_asb = [None] * QT
        pending_norm = [[]]

        def attention_head(h, inner=None):
            g, a = h // 4, h % 4
            p0 = 32 * a
            av = [
                pav.tile([128, 4, 65], F32, tag="av", name=f"av{h}_{i}")
                for i in range(2)
            ]

            def av_half(pt, kt, hv):
                for q4 in range(4):
                    qt = 4 * hv + q4
                    nc.tensor.matmul(
                        av[hv][:, q4, :],
                        lhsT=pt[:, qt * 128 : (qt + 1) * 128],
                        rhs=VO[:, kt, h, :],
                        start=(kt == 0 and q4 == 0),
                        stop=(kt == KT - 1),
                        skip_group_check=True,
                    )

            # AV for tile kt is emitted 1 slot late (av half 0) / 2 slots
            # late (half 1): the exp-wait then never head-of-line blocks the
            # next scores, and the head-boundary av-tile reuse (pav bufs=2)
            # has time to clear through the deferred normalize jobs.
            hist = []
            for kt in range(KT):
                ps_s = pps.tile([128, NQ], F32, tag="ps_s")
                for qh in range(2):
                    nc.tensor.matmul(
                        ps_s[:, qh * 512 : (qh + 1) * 512],
                        lhsT=K8[g][p0 : p0 + 32, :, kt * 128 : (kt + 1) * 128],
                        rhs=Q8[g][p0 : p0 + 32, :, qh * 512 : (qh + 1) * 512],
                        start=True,
                        stop=True,
                        perf_mode=DR,
                        tile_position=(p0, 0),
                    )
                pt = ppt.tile([128, NQ], BF16, tag="pt")
                # head 0: DVE is busy producing the fp8 x copy; keep its
                # whole exp stream on the scalar engine
                if kt in DVE_KT and h != 0:
                    nc.vector.tensor_scalar(
                        out=pt.bitcast(I16), in0=ps_s,
                        scalar1=S16_MUL, scalar2=S16_ADD,
                        op0=ALU.mult, op1=ALU.add,
                    )
                else:
                    nc.scalar.activation(pt, ps_s, AF.Exp, scale=0.125)
                if len(hist) >= 1:
                    av_half(hist[-1][0], hist[-1][1], 0)
                if len(hist) >= 2:
                    av_half(hist[-2][0], hist[-2][1], 1)
                if inner is not None and kt < len(inner):
                    inner[kt]()
                hist = hist[-1:] + [(pt, kt)]
            av_half(hist[-2][0], hist[-2][1], 1)
            av_half(hist[-1][0], hist[-1][1], 0)
            av_half(hist[-1][0], hist[-1][1], 1)

            # normalize (reciprocal + scale + AT transpose) is deferred into
            # the next head's first slots so it never stalls the exp stream.
            hp = h // 2
            rec_box = {}

            def mk_norm(qt):
                def job():
                    if qt % 4 == 0:
                        # one reciprocal covers the 4 denominators of av[hv]
                        rec_box[qt // 4] = prec.tile(
                            [128, 4, 1], F32, tag="rec", name=f"rec{h}_{qt // 4}"
                        )
                        nc.vector.reciprocal_approx_fast(
                            rec_box[qt // 4], av[qt // 4][:, :, 64:65]
                        )
                    if h % 2 == 0:
                        cur_asb[qt] = pasb.tile(
                            [128, 128], BF16, tag="asb", name=f"asb{h}_{qt}"
                        )
                    nc.vector.tensor_scalar_mul(
                        cur_asb[qt][:, 64 * (h % 2) : 64 * (h % 2) + 64],
                        av[qt // 4][:, qt % 4, 0:64],
                        rec_box[qt // 4][:, qt % 4, :],
                    )
                    if h % 2 == 1:
                        nc.sync.dma_start(
                            out=AT[hp][:, qt * 128 : (qt + 1) * 128],
                            in_=cur_asb[qt],
                            transpose=True,
                        )

                return job

            pending_norm[0] = [mk_norm(qt) for qt in range(QT)]

        for h in range(H):
            if h == 11:
                # all Q/K chains are emitted; reuse xT8/w8 space for wobf
                # and the out-proj staging buffers
                pearly_es.close()
                plate_box[0] = es.enter_context(tc.tile_pool(name="late", bufs=1))
                posb_box[0] = es.enter_context(tc.tile_pool(name="osb", bufs=4))
            norms = pending_norm[0]
            pending_norm[0] = []
            norm_jobs = [
                (lambda grp=norms[i : i + 4]: [j() for j in grp])
                for i in range(0, len(norms), 4)
            ]
            window_jobs = jobs.get(h, [])
            inner = norm_jobs + [(lambda j=j: j()) for j in window_jobs]
            attention_head(h, inner=inner)

        # --- tail: last head's normalize, then out-proj pass B2 ----------
        for j in pending_norm[0]:
            j()
        for tt in range(8):
            outproj_passB2(tt)


_NC = None


def _get_program():
    global _NC
    if _NC is None:
        _NC = build_program()
    return _NC


def make_in_maps(x, Wqkv, bqkv, Wo, bo):
    f8 = np.dtype(mybir.dt.np(F8))
    bf16 = np.dtype(mybir.dt.np(BF16))
    Wqkv = np.asarray(Wqkv, np.float32)
    w8 = (Wqkv[:, : 2 * E] * WSCALE).reshape(4, 2, 128, 2 * E)
    w8 = np.ascontiguousarray(w8.transpose(2, 0, 1, 3)).astype(f8)
    wvbf = Wqkv[:, 2 * E :].reshape(8, 128, E)
    wvbf = np.ascontiguousarray(wvbf.transpose(1, 0, 2)).astype(bf16)
    wobf = np.asarray(Wo, np.float32).reshape(8, 128, E)
    wobf = np.ascontiguousarray(wobf.transpose(1, 0, 2)).astype(bf16)
    w = {
        "w8": w8,
        "wvbf": wvbf,
        "wobf": wobf,
        "bqkv": np.ascontiguousarray(np.asarray(bqkv, np.float32)),
        "bo": np.ascontiguousarray(np.asarray(bo, np.float32)),
    }
    x = np.asarray(x, np.float32)
    in_maps = []
    for c in range(N_CORES):
        b, s = divmod(c, 2)
        xb = x[b]
        if s == 1:
            xb = np.roll(xb, -NQ, axis=0)
        xTb = np.ascontiguousarray(
            xb.T.reshape(8, 128, SEQ).transpose(1, 0, 2)
        ).astype(bf16)
        in_maps.append({"xTb": xTb, **w})
    return in_maps


def gather_out(results):
    out = np.empty((4, SEQ, E), np.float32)
    for c in range(N_CORES):
        b, s = divmod(c, 2)
        out[b, s * NQ : (s + 1) * NQ] = results[c]["out"]
    return out


def kernel(x, Wqkv, bqkv, Wo, bo):
    from concourse.bass_utils import run_bass_kernel_spmd

    nc = _get_program()
    in_maps = make_in_maps(x, Wqkv, bqkv, Wo, bo)
    res = run_bass_kernel_spmd(nc, in_maps, core_ids=list(range(N_CORES)))
    return gather_out(res.results)
